# revision 4
# baseline (speedup 1.0000x reference)
"""AdaLoRA routed-LoRA kernel for 8 Trainium2 NeuronCores.

Problem (nn_AdaLoRA): per token t with expert index i:
    ds[t, :]  = slots[t, :] @ down_table[i]          # [1024] @ [1024, 16]
    out[t, :] = (ds[t, :] @ up_table[i]) * 1/sqrt(16)  # [16] @ [16, 1024]

Sharding: data-parallel over batch (B=8 -> one batch row per core; LoRA
tables replicated on every core). Per core: 256 tokens = 2 tiles of 128
tokens (tokens on SBUF partitions).

Per tile: indirect-DMA gather each token's 64KB down row and 64KB up row
into its partition (chunked for SBUF streaming), then:
  down-proj: scalar_tensor_tensor(mult, mult) with accum_out -> fused
             multiply+reduce per rank (in1 = rank-r slice, stride 16)
  up-proj:   scalar_tensor_tensor(mult, add) with per-partition scalar
             ds[:, r] -> fused multiply-accumulate per rank
  scale:     folded in on the scalar engine before the output DMA.
"""

import numpy as np

B, K, DIM, RANK, NE = 8, 256, 1024, 16, 4096
ROW = DIM * RANK  # 16384 elements per table row
SCALE = 1.0 / 4.0  # 1/sqrt(RANK)
P = 128
N_TILE = K // P  # 2 token tiles per core
DCH = 2  # down-table chunks per tile (512 d-values each)
UCH = 2  # up-table chunks per tile (8 ranks each)
N_CORES = 8

_CACHE = {}
DEBUG_VARIANT = "full"  # full | noact | nogather | nocompute


def _build():
    from concourse import bacc, bass, mybir, tile

    f32 = mybir.dt.float32
    i32 = mybir.dt.int32
    mult = mybir.AluOpType.mult
    add = mybir.AluOpType.add

    nc = bacc.Bacc("TRN2", target_bir_lowering=False)
    slots = nc.declare_dram_parameter("slots", [K, DIM], f32, isOutput=False)
    idx = nc.declare_dram_parameter("idx", [K, 1], i32, isOutput=False)
    down = nc.declare_dram_parameter("down", [NE, ROW], f32, isOutput=False)
    up = nc.declare_dram_parameter("up", [NE, ROW], f32, isOutput=False)
    out = nc.declare_dram_parameter("out", [K, DIM], f32, isOutput=True)

    DC = DIM // DCH  # 512 d per down chunk
    RC = RANK // UCH  # 8 ranks per up chunk

    with tile.TileContext(nc) as tc:
        with (
            tc.tile_pool(name="io", bufs=2) as io_pool,
            tc.tile_pool(name="gather", bufs=2) as gpool,
            tc.tile_pool(name="misc", bufs=1) as mpool,
        ):
            scratch = mpool.tile([P, DC], f32)
            for t in range(N_TILE):
                tok = slice(t * P, (t + 1) * P)
                idx_t = io_pool.tile([P, 1], i32, tag="idx")
                nc.sync.dma_start(out=idx_t[:], in_=idx[tok, :])
                slots_t = io_pool.tile([P, DIM], f32, tag="slots")
                nc.sync.dma_start(out=slots_t[:], in_=slots[tok, :])

                # ---- down projection: ds[t, r] = sum_d slots[t,d]*down_i[d,r]
                dsp = []
                for c in range(DCH):
                    dch = gpool.tile([P, DC, RANK], f32, tag="dch")
                    if DEBUG_VARIANT == "nogather":
                        nc.sync.dma_start(
                            out=dch[:],
                            in_=down[0 : 2 * P : 2, c * DC * RANK : (c + 1) * DC * RANK].rearrange(
                                "p (d r) -> p d r", r=RANK
                            ),
                        )
                    else:
                        nc.gpsimd.indirect_dma_start(
                            out=dch[:].rearrange("p d r -> p (d r)"),
                            out_offset=None,
                            in_=down[:],
                            in_offset=bass.IndirectOffsetOnAxis(ap=idx_t[:, :1], axis=0),
                            element_offset=c * DC * RANK,
                        )
                    dsp_c = io_pool.tile([P, RANK], f32, tag=f"dsp{c}")
                    if DEBUG_VARIANT == "nocompute":
                        nc.vector.memset(dsp_c[:], 0.0)
                        dsp.append(dsp_c)
                        continue
                    for r in range(RANK):
                        nc.vector.scalar_tensor_tensor(
                            out=scratch[:],
                            in0=slots_t[:, c * DC : (c + 1) * DC],
                            scalar=1.0,
                            in1=dch[:, :, r],
                            op0=mult,
                            op1=mult,
                            accum_out=dsp_c[:, r : r + 1],
                        )
                    dsp.append(dsp_c)
                ds = io_pool.tile([P, RANK], f32, tag="ds")
                nc.vector.tensor_tensor(
                    out=ds[:], in0=dsp[0][:], in1=dsp[1][:], op=add
                )

                # ---- up projection: out[t, d] = sum_r ds[t,r]*up_i[r,d]
                acc = io_pool.tile([P, DIM], f32, tag="acc")
                for c in range(UCH):
                    uch = gpool.tile([P, RC, DIM], f32, tag="uch")
                    if DEBUG_VARIANT == "nogather":
                        nc.sync.dma_start(
                            out=uch[:],
                            in_=up[0 : 2 * P : 2, c * RC * DIM : (c + 1) * RC * DIM].rearrange(
                                "p (r d) -> p r d", d=DIM
                            ),
                        )
                    else:
                        nc.gpsimd.indirect_dma_start(
                            out=uch[:].rearrange("p r d -> p (r d)"),
                            out_offset=None,
                            in_=up[:],
                            in_offset=bass.IndirectOffsetOnAxis(ap=idx_t[:, :1], axis=0),
                            element_offset=c * RC * DIM,
                        )
                    for j in range(RC):
                        r = c * RC + j
                        if DEBUG_VARIANT == "nocompute":
                            if r == 0:
                                nc.vector.tensor_copy(out=acc[:], in_=uch[:, 0, :])
                            continue
                        if r == 0:
                            nc.vector.tensor_scalar(
                                out=acc[:],
                                in0=uch[:, 0, :],
                                scalar1=ds[:, 0:1],
                                scalar2=None,
                                op0=mult,
                            )
                        else:
                            nc.vector.scalar_tensor_tensor(
                                out=acc[:],
                                in0=uch[:, j, :],
                                scalar=ds[:, r : r + 1],
                                in1=acc[:],
                                op0=mult,
                                op1=add,
                            )
                # fold the 1/sqrt(RANK) scale in
                if DEBUG_VARIANT == "full":
                    nc.scalar.mul(acc[:], acc[:], SCALE)
                else:
                    nc.vector.tensor_scalar_mul(out=acc[:], in0=acc[:], scalar1=SCALE)
                nc.sync.dma_start(out=out[tok, :], in_=acc[:])
    nc.compile()
    return nc


def _get_nc():
    if "nc" not in _CACHE:
        _CACHE["nc"] = _build()
    return _CACHE["nc"]


def _prep_in_maps(slots, indices, down_proj_values, up_proj_values):
    slots = np.ascontiguousarray(np.asarray(slots, dtype=np.float32))
    indices = np.asarray(indices)
    indices = np.ascontiguousarray(indices.astype(np.int32))
    down = np.ascontiguousarray(
        np.asarray(down_proj_values, dtype=np.float32).reshape(NE, ROW)
    )
    up = np.ascontiguousarray(
        np.asarray(up_proj_values, dtype=np.float32).reshape(NE, ROW)
    )
    assert slots.shape == (B, K, DIM) and indices.shape == (B, K)
    in_maps = []
    for i in range(N_CORES):
        in_maps.append(
            {
                "slots": slots[i],
                "idx": indices[i].reshape(K, 1),
                "down": down,
                "up": up,
            }
        )
    return in_maps


def _run(in_maps, trace=False):
    from concourse.bass_utils import run_bass_kernel_spmd

    nc = _get_nc()
    return run_bass_kernel_spmd(
        nc, in_maps, core_ids=list(range(N_CORES)), trace=trace
    )


def kernel(slots, indices, down_proj_values, up_proj_values):
    in_maps = _prep_in_maps(slots, indices, down_proj_values, up_proj_values)
    res = _run(in_maps)
    out = np.stack([res.results[i]["out"] for i in range(N_CORES)], axis=0)
    return out.astype(np.float32)
